# revision 8
# baseline (speedup 1.0000x reference)
"""Trainium2 Bass kernel for nn_CiderFeatures (all-pairs Gaussian reduction).

y[i, c] = norms[c] * sum_j exp(-(a_j + b[i,c]) * ||x_i - x_j||^2) * f_j

with per-point scalars a, b, f derived from (rho, gamma, weights).

Strategy (8 NeuronCores, row-parallel over i):
  - The exp argument is a bilinear form: arg[ic, j] = V[:, ic] . U[:, j]
    with 10 logical contraction dims (the expansion of
    -(a_j + b_ic) (r_i + r_j - 2 x_i.x_j) + ln f_j + ln norms_c).
  - fp32 matmuls run at 1/4 rate on the PE, so each logical dim is split
    into bf16 hi/mid/lo levels and the cross products are stacked into the
    contraction dim (K~50..90).  bf16*bf16 products are exact in fp32
    accumulation, recovering ~fp32 precision at full PE speed.
  - TensorE computes arg tiles [128 ic, 512 j] into PSUM; ScalarE (ACT)
    computes exp and the j-sum in one pass via accum_out; VectorE adds the
    per-chunk partial sums.  ACT is the bottleneck (~3N^2/8 exps per core).
"""

import numpy as np
import ml_dtypes
from math import pi

N = 16384
N_CORES = 8
ROWS_PER_CORE = N // N_CORES          # 2048
IC_PER_CORE = 3 * ROWS_PER_CORE       # 6144
BLOCKS_PER_CORE = IC_PER_CORE // 128  # 48
J_GROUP = 2048                        # PSUM tile free size (4 banks)
N_GROUPS = N // J_GROUP               # 8
MM_N = 512                            # one PSUM bank of fp32
LNF_FLOOR = -100.0                    # ln f clamp for f == 0

# number of bf16 levels per operand and max level-sum kept
SPLIT_LEVELS = 3
MAX_LEVEL_SUM = 2


def _derived(rho, gamma, weights, coords):
    """Per-point scalars, computed in float64 (mirrors reference fp32 math)."""
    A, D = 2.0, 2.0
    B2, C2 = A, (6.0 * pi ** 2) ** (2.0 / 3.0) * (6.0 * A / (160.0 * pi))
    B3, C3 = 2.0 * B2, 2.0 * C2
    B0, C0 = D / A * B2, D / A * C2
    B1, C1 = B2 / 2.0, C2 / 2.0
    Bs = np.array([B0, B1, B2, B3])
    Cs = np.array([C0, C1, C2, C3])
    norms = ((Bs[0] + Bs[1:]) / 2.0) ** 1.5  # (3,)

    rho_ = rho + 1e-8
    t_w = gamma / (8.0 * rho_)
    t_tf = 0.3 * (3.0 * pi ** 2) ** (2.0 / 3.0) * rho_ ** (5.0 / 3.0)
    x = t_w / t_tf
    scale = pi * (rho_ / 2.0) ** (2.0 / 3.0)
    ab = scale[:, None] * (Bs[None, :] + Cs[None, :] * x[:, None])  # (N,4)
    a = ab[:, 0]
    b = ab[:, 1:]                                                   # (N,3)
    f = weights * rho
    lnf = np.log(np.maximum(f, 1e-300))
    lnf = np.maximum(lnf, LNF_FLOOR)
    r = np.sum(coords * coords, axis=1)                             # (N,)
    return a, b, f, lnf, r, norms


def _build_vu10(rho, gamma, coords, weights):
    """The 10-dim bilinear decomposition (float64).

    Returns V10 [10, N, 3] (per (i, c)) and U10 [10, N] (per j) with
      arg[ic, j] = sum_k V10[k, i, c] * U10[k, j]
                 = -(a_j + b_ic) * ||x_i - x_j||^2 + ln f_j + ln norms_c
    a and r are mean-centered to shrink cross-product magnitudes (the
    centered remainders fold into the pure-i / pure-j dims exactly).
    """
    a, b, f, lnf, r, norms = _derived(rho, gamma, weights, coords)
    lnn = np.log(norms)                                   # (3,)
    rbar = float(r.mean())
    rc = r - rbar
    abar = float(a.mean())
    ac = a - abar
    xyz = coords                                          # (N, 3)

    V10 = np.empty((10, N, 3))
    U10 = np.empty((10, N))

    # dim0: cross  -ac_j * rc_i
    V10[0] = np.broadcast_to(rc[:, None], (N, 3))
    U10[0] = -ac
    # dim1: pure j  (-a_j r_j + lnf_j - ac_j rbar)
    V10[1] = 1.0
    U10[1] = -a * r + lnf - ac * rbar
    # dims2-4: cross  2 x_i . (ac_j x_j)
    V10[2:5] = np.broadcast_to((2.0 * xyz).T[:, :, None], (3, N, 3))
    U10[2:5] = (ac[:, None] * xyz).T
    # dim5: cross  -b_ic * rc_j
    V10[5] = b
    U10[5] = -rc
    # dim6: pure ic  (-b_ic (r_i + rbar) - abar (rc_i + rbar) + lnn_c)
    V10[6] = (-(b * (r[:, None] + rbar))
              - abar * (rc[:, None] + rbar)
              + lnn[None, :])
    U10[6] = 1.0
    # dims7-9: cross  2 (b_ic + abar) x_i . x_j
    V10[7:10] = np.moveaxis(
        2.0 * (b + abar)[:, :, None] * xyz[:, None, :], 2, 0)
    U10[7:10] = xyz.T
    return V10, U10


def _bf16_levels(M, nlev):
    """Split float64 array into bf16-representable float64 level arrays."""
    rem = M.copy()
    outs = []
    for _ in range(nlev):
        h = np.asarray(rem, ml_dtypes.bfloat16).astype(np.float64)
        outs.append(h)
        rem = rem - h
    return outs


def build_split_vu(rho, gamma, coords, weights,
                   nlev=SPLIT_LEVELS, max_sum=MAX_LEVEL_SUM):
    """Build the bf16-split V/U matrices.

    Returns (Vb [K, N, 3], Ub [K, N]) float32 arrays whose values are
    bf16-representable, with  arg ~= sum_k Vb[k] * Ub[k]  accumulated in
    fp32.  Rows are ordered by level-sum (hi*hi products first).
    """
    V10, U10 = _build_vu10(rho, gamma, coords, weights)
    Vlev = [_bf16_levels(V10[d], nlev) for d in range(10)]
    Ulev = [_bf16_levels(U10[d], nlev) for d in range(10)]

    vrows, urows = [], []
    for s in range(max_sum + 1):
        for d in range(10):
            for lv in range(min(s, nlev - 1) + 1):
                lu = s - lv
                if lu >= nlev:
                    continue
                v = Vlev[d][lv]
                u = Ulev[d][lu]
                if not v.any() or not u.any():
                    continue
                vrows.append(v)
                urows.append(u)
    Vb = np.stack(vrows).astype(np.float32)   # [K, N, 3]
    Ub = np.stack(urows).astype(np.float32)   # [K, N]
    return Vb, Ub


# ---------------------------------------------------------------------------
# Device kernel
# ---------------------------------------------------------------------------

_NC_CACHE = {}


def _build_nc(KK, repeat=1):
    """One-core Bass program (SPMD across 8 cores with per-core inputs).

    repeat > 1 re-runs the whole compute loop (for benchmarking slope)."""
    import concourse.bass as bass  # noqa: F401
    import concourse.tile as tile
    from concourse import bacc, mybir

    nc = bacc.Bacc("TRN2", target_bir_lowering=False)
    u_dram = nc.dram_tensor("u", [KK, N], mybir.dt.bfloat16,
                            kind="ExternalInput")
    v_dram = nc.dram_tensor("v", [KK, IC_PER_CORE], mybir.dt.bfloat16,
                            kind="ExternalInput")
    y_dram = nc.dram_tensor("y", [128, BLOCKS_PER_CORE], mybir.dt.float32,
                            kind="ExternalOutput")

    # groups whose j-reduction runs on VectorE (reading fp32 exp scratch)
    # instead of ACT accum_out; balances ACT vs DVE occupancy.
    DVE_GROUPS = frozenset((0, 1, 2, 4, 5, 6))

    with tile.TileContext(nc) as tc:
        with (
            tc.tile_pool(name="singles", bufs=1) as singles,
            tc.tile_pool(name="upool", bufs=N_GROUPS) as upool,
            tc.tile_pool(name="psum", bufs=2, space="PSUM") as psum_pool,
            tc.tile_pool(name="scratch", bufs=3) as scratch_pool,
            tc.tile_pool(name="parts", bufs=3) as parts_pool,
        ):
            # warm the ACT exp table during the input-DMA window
            warm = singles.tile([128, 1], mybir.dt.float32)
            nc.vector.memset(warm[:], 0.0)
            nc.scalar.activation(out=warm[:], in_=warm[:],
                                 func=mybir.ActivationFunctionType.Exp)

            v_sb = singles.tile([KK, IC_PER_CORE], mybir.dt.bfloat16)
            nc.sync.dma_start(v_sb[:], v_dram[:])
            u_tiles = []
            for g in range(N_GROUPS):
                ut = upool.tile([KK, J_GROUP], mybir.dt.bfloat16, tag="u")
                nc.sync.dma_start(ut[:], u_dram[:, g * J_GROUP:(g + 1) * J_GROUP])
                u_tiles.append(ut)
            y_sb = singles.tile([128, BLOCKS_PER_CORE], mybir.dt.float32)

            for B in [b for _ in range(repeat) for b in range(BLOCKS_PER_CORE)]:
                lhsT = v_sb[:, B * 128:(B + 1) * 128]
                parts = parts_pool.tile([128, N_GROUPS], mybir.dt.float32,
                                        tag="parts")
                for g in range(N_GROUPS):
                    pt = psum_pool.tile([128, J_GROUP], mybir.dt.float32,
                                        tag="ps")
                    for q in range(J_GROUP // MM_N):
                        nc.tensor.matmul(
                            pt[:, q * MM_N:(q + 1) * MM_N],
                            lhsT,
                            u_tiles[g][:, q * MM_N:(q + 1) * MM_N],
                            start=True, stop=True)
                    if g in DVE_GROUPS:
                        sc = scratch_pool.tile([128, J_GROUP],
                                               mybir.dt.float32, tag="sc")
                        nc.scalar.activation(
                            out=sc[:], in_=pt[:],
                            func=mybir.ActivationFunctionType.Exp)
                        nc.vector.reduce_sum(parts[:, g:g + 1], sc[:],
                                             axis=mybir.AxisListType.X)
                    else:
                        # exp in place in PSUM (ScalarE's cheapest port),
                        # j-sum via the ACT accumulator
                        nc.scalar.activation(
                            out=pt[:], in_=pt[:],
                            func=mybir.ActivationFunctionType.Exp,
                            accum_out=parts[:, g:g + 1])
                nc.vector.reduce_sum(y_sb[:, B:B + 1], parts[:],
                                     axis=mybir.AxisListType.X)
            nc.sync.dma_start(y_dram[:], y_sb[:])
    nc.finalize()
    return nc


def _prep_inputs(rho, gamma, coords, weights):
    rho = np.asarray(rho, np.float64)
    gamma = np.asarray(gamma, np.float64)
    coords = np.asarray(coords, np.float64)
    weights = np.asarray(weights, np.float64)
    Vb, Ub = build_split_vu(rho, gamma, coords, weights)
    KK = Vb.shape[0]
    Ub16 = np.ascontiguousarray(Ub.astype(ml_dtypes.bfloat16))
    in_maps = []
    for m in range(N_CORES):
        vc = Vb[:, m * ROWS_PER_CORE:(m + 1) * ROWS_PER_CORE, :]  # [K, 2048, 3]
        vc = np.moveaxis(vc, 2, 1).reshape(KK, IC_PER_CORE)       # c-major cols
        in_maps.append({"u": Ub16,
                        "v": np.ascontiguousarray(vc.astype(ml_dtypes.bfloat16))})
    return KK, in_maps


def _assemble(results):
    out = np.empty((N, 3), np.float32)
    for m, res in enumerate(results):
        y_dev = np.asarray(res["y"])                   # [128, 48]
        flat = y_dev.T.reshape(IC_PER_CORE)            # ic = B*128 + p order
        out[m * ROWS_PER_CORE:(m + 1) * ROWS_PER_CORE, :] = (
            flat.reshape(3, ROWS_PER_CORE).T)
    return out


def kernel_run(rho, gamma, coords, weights, **spmd_kwargs):
    """Run on hardware; returns (y, BassKernelResults)."""
    from concourse.bass_utils import run_bass_kernel_spmd

    KK, in_maps = _prep_inputs(rho, gamma, coords, weights)
    if KK not in _NC_CACHE:
        _NC_CACHE[KK] = _build_nc(KK)
    res = run_bass_kernel_spmd(_NC_CACHE[KK], in_maps,
                               core_ids=list(range(N_CORES)), **spmd_kwargs)
    return _assemble(res.results), res


def kernel(rho, gamma, coords, weights):
    y, _ = kernel_run(rho, gamma, coords, weights)
    return y


# revision 10
# speedup vs baseline: 1.0030x; 1.0030x over previous
"""Trainium2 Bass kernel for nn_CiderFeatures (all-pairs Gaussian reduction).

y[i, c] = norms[c] * sum_j exp(-(a_j + b[i,c]) * ||x_i - x_j||^2) * f_j

with per-point scalars a, b, f derived from (rho, gamma, weights).

Strategy (8 NeuronCores, row-parallel over i):
  - The exp argument is a bilinear form: arg[ic, j] = V[:, ic] . U[:, j]
    with 10 logical contraction dims (the expansion of
    -(a_j + b_ic) (r_i + r_j - 2 x_i.x_j) + ln f_j + ln norms_c).
  - fp32 matmuls run at 1/4 rate on the PE, so each logical dim is split
    into bf16 hi/mid/lo levels and the cross products are stacked into the
    contraction dim (K~50..90).  bf16*bf16 products are exact in fp32
    accumulation, recovering ~fp32 precision at full PE speed.
  - TensorE computes arg tiles [128 ic, 512 j] into PSUM; ScalarE (ACT)
    computes exp and the j-sum in one pass via accum_out; VectorE adds the
    per-chunk partial sums.  ACT is the bottleneck (~3N^2/8 exps per core).
"""

import numpy as np
import ml_dtypes
from math import pi

N = 16384
N_CORES = 8
ROWS_PER_CORE = N // N_CORES          # 2048
IC_PER_CORE = 3 * ROWS_PER_CORE       # 6144
BLOCKS_PER_CORE = IC_PER_CORE // 128  # 48
J_GROUP = 2048                        # PSUM tile free size (4 banks)
N_GROUPS = N // J_GROUP               # 8
MM_N = 512                            # one PSUM bank of fp32
LNF_FLOOR = -100.0                    # ln f clamp for f == 0

# number of bf16 levels per operand and max level-sum kept
SPLIT_LEVELS = 3
MAX_LEVEL_SUM = 2


def _derived(rho, gamma, weights, coords):
    """Per-point scalars, computed in float64 (mirrors reference fp32 math)."""
    A, D = 2.0, 2.0
    B2, C2 = A, (6.0 * pi ** 2) ** (2.0 / 3.0) * (6.0 * A / (160.0 * pi))
    B3, C3 = 2.0 * B2, 2.0 * C2
    B0, C0 = D / A * B2, D / A * C2
    B1, C1 = B2 / 2.0, C2 / 2.0
    Bs = np.array([B0, B1, B2, B3])
    Cs = np.array([C0, C1, C2, C3])
    norms = ((Bs[0] + Bs[1:]) / 2.0) ** 1.5  # (3,)

    rho_ = rho + 1e-8
    t_w = gamma / (8.0 * rho_)
    t_tf = 0.3 * (3.0 * pi ** 2) ** (2.0 / 3.0) * rho_ ** (5.0 / 3.0)
    x = t_w / t_tf
    scale = pi * (rho_ / 2.0) ** (2.0 / 3.0)
    ab = scale[:, None] * (Bs[None, :] + Cs[None, :] * x[:, None])  # (N,4)
    a = ab[:, 0]
    b = ab[:, 1:]                                                   # (N,3)
    f = weights * rho
    lnf = np.log(np.maximum(f, 1e-300))
    lnf = np.maximum(lnf, LNF_FLOOR)
    r = np.sum(coords * coords, axis=1)                             # (N,)
    return a, b, f, lnf, r, norms


def _build_vu10(rho, gamma, coords, weights):
    """The 10-dim bilinear decomposition (float64).

    Returns V10 [10, N, 3] (per (i, c)) and U10 [10, N] (per j) with
      arg[ic, j] = sum_k V10[k, i, c] * U10[k, j]
                 = -(a_j + b_ic) * ||x_i - x_j||^2 + ln f_j + ln norms_c
    a and r are mean-centered to shrink cross-product magnitudes (the
    centered remainders fold into the pure-i / pure-j dims exactly).
    """
    a, b, f, lnf, r, norms = _derived(rho, gamma, weights, coords)
    lnn = np.log(norms)                                   # (3,)
    rbar = float(r.mean())
    rc = r - rbar
    abar = float(a.mean())
    ac = a - abar
    xyz = coords                                          # (N, 3)

    V10 = np.empty((10, N, 3))
    U10 = np.empty((10, N))

    # dim0: cross  -ac_j * rc_i
    V10[0] = np.broadcast_to(rc[:, None], (N, 3))
    U10[0] = -ac
    # dim1: pure j  (-a_j r_j + lnf_j - ac_j rbar)
    V10[1] = 1.0
    U10[1] = -a * r + lnf - ac * rbar
    # dims2-4: cross  2 x_i . (ac_j x_j)
    V10[2:5] = np.broadcast_to((2.0 * xyz).T[:, :, None], (3, N, 3))
    U10[2:5] = (ac[:, None] * xyz).T
    # dim5: cross  -b_ic * rc_j
    V10[5] = b
    U10[5] = -rc
    # dim6: pure ic  (-b_ic (r_i + rbar) - abar (rc_i + rbar) + lnn_c)
    V10[6] = (-(b * (r[:, None] + rbar))
              - abar * (rc[:, None] + rbar)
              + lnn[None, :])
    U10[6] = 1.0
    # dims7-9: cross  2 (b_ic + abar) x_i . x_j
    V10[7:10] = np.moveaxis(
        2.0 * (b + abar)[:, :, None] * xyz[:, None, :], 2, 0)
    U10[7:10] = xyz.T
    return V10, U10


def _bf16_levels(M, nlev):
    """Split float64 array into bf16-representable float64 level arrays."""
    rem = M.copy()
    outs = []
    for _ in range(nlev):
        h = np.asarray(rem, ml_dtypes.bfloat16).astype(np.float64)
        outs.append(h)
        rem = rem - h
    return outs


def build_split_vu(rho, gamma, coords, weights,
                   nlev=SPLIT_LEVELS, max_sum=MAX_LEVEL_SUM):
    """Build the bf16-split V/U matrices.

    Returns (Vb [K, N, 3], Ub [K, N]) float32 arrays whose values are
    bf16-representable, with  arg ~= sum_k Vb[k] * Ub[k]  accumulated in
    fp32.  Rows are ordered by level-sum (hi*hi products first).
    """
    V10, U10 = _build_vu10(rho, gamma, coords, weights)
    Vlev = [_bf16_levels(V10[d], nlev) for d in range(10)]
    Ulev = [_bf16_levels(U10[d], nlev) for d in range(10)]

    vrows, urows = [], []
    for s in range(max_sum + 1):
        for d in range(10):
            for lv in range(min(s, nlev - 1) + 1):
                lu = s - lv
                if lu >= nlev:
                    continue
                v = Vlev[d][lv]
                u = Ulev[d][lu]
                if not v.any() or not u.any():
                    continue
                vrows.append(v)
                urows.append(u)
    Vb = np.stack(vrows).astype(np.float32)   # [K, N, 3]
    Ub = np.stack(urows).astype(np.float32)   # [K, N]
    return Vb, Ub


# ---------------------------------------------------------------------------
# Device kernel
# ---------------------------------------------------------------------------

_NC_CACHE = {}


def _build_nc(KK, repeat=1):
    """One-core Bass program (SPMD across 8 cores with per-core inputs).

    repeat > 1 re-runs the whole compute loop (for benchmarking slope)."""
    import concourse.bass as bass  # noqa: F401
    import concourse.tile as tile
    from concourse import bacc, mybir

    nc = bacc.Bacc("TRN2", target_bir_lowering=False)
    u_dram = nc.dram_tensor("u", [KK, N], mybir.dt.bfloat16,
                            kind="ExternalInput")
    v_dram = nc.dram_tensor("v", [KK, IC_PER_CORE], mybir.dt.bfloat16,
                            kind="ExternalInput")
    y_dram = nc.dram_tensor("y", [128, BLOCKS_PER_CORE], mybir.dt.float32,
                            kind="ExternalOutput")

    # groups whose j-reduction runs on VectorE (reading fp32 exp scratch)
    # instead of ACT accum_out; alternating 6/7 between blocks balances the
    # ACT and DVE engine-busy times (both ~93% occupied).
    DVE_GROUPS_EVEN = frozenset((0, 1, 2, 4, 5, 6))
    DVE_GROUPS_ODD = frozenset((0, 1, 2, 3, 4, 5, 6))

    with tile.TileContext(nc) as tc:
        with (
            tc.tile_pool(name="singles", bufs=1) as singles,
            tc.tile_pool(name="upool", bufs=N_GROUPS) as upool,
            tc.tile_pool(name="psum", bufs=2, space="PSUM") as psum_pool,
            tc.tile_pool(name="scratch", bufs=3) as scratch_pool,
            tc.tile_pool(name="parts", bufs=3) as parts_pool,
        ):
            # warm the ACT exp table during the input-DMA window
            warm = singles.tile([128, 1], mybir.dt.float32)
            nc.vector.memset(warm[:], 0.0)
            nc.scalar.activation(out=warm[:], in_=warm[:],
                                 func=mybir.ActivationFunctionType.Exp)

            v_sb = singles.tile([KK, IC_PER_CORE], mybir.dt.bfloat16)
            nc.sync.dma_start(v_sb[:], v_dram[:])
            u_tiles = []
            for g in range(N_GROUPS):
                ut = upool.tile([KK, J_GROUP], mybir.dt.bfloat16, tag="u")
                nc.sync.dma_start(ut[:], u_dram[:, g * J_GROUP:(g + 1) * J_GROUP])
                u_tiles.append(ut)
            y_sb = singles.tile([128, BLOCKS_PER_CORE], mybir.dt.float32)

            for B in [b for _ in range(repeat) for b in range(BLOCKS_PER_CORE)]:
                lhsT = v_sb[:, B * 128:(B + 1) * 128]
                dve_groups = DVE_GROUPS_EVEN if B % 2 == 0 else DVE_GROUPS_ODD
                parts = parts_pool.tile([128, N_GROUPS], mybir.dt.float32,
                                        tag="parts")
                for g in range(N_GROUPS):
                    pt = psum_pool.tile([128, J_GROUP], mybir.dt.float32,
                                        tag="ps")
                    for q in range(J_GROUP // MM_N):
                        nc.tensor.matmul(
                            pt[:, q * MM_N:(q + 1) * MM_N],
                            lhsT,
                            u_tiles[g][:, q * MM_N:(q + 1) * MM_N],
                            start=True, stop=True)
                    if g in dve_groups:
                        sc = scratch_pool.tile([128, J_GROUP],
                                               mybir.dt.float32, tag="sc")
                        nc.scalar.activation(
                            out=sc[:], in_=pt[:],
                            func=mybir.ActivationFunctionType.Exp)
                        nc.vector.reduce_sum(parts[:, g:g + 1], sc[:],
                                             axis=mybir.AxisListType.X)
                    else:
                        # exp in place in PSUM (ScalarE's cheapest port),
                        # j-sum via the ACT accumulator
                        nc.scalar.activation(
                            out=pt[:], in_=pt[:],
                            func=mybir.ActivationFunctionType.Exp,
                            accum_out=parts[:, g:g + 1])
                nc.vector.reduce_sum(y_sb[:, B:B + 1], parts[:],
                                     axis=mybir.AxisListType.X)
            nc.sync.dma_start(y_dram[:], y_sb[:])
    nc.finalize()
    return nc


def _prep_inputs(rho, gamma, coords, weights):
    rho = np.asarray(rho, np.float64)
    gamma = np.asarray(gamma, np.float64)
    coords = np.asarray(coords, np.float64)
    weights = np.asarray(weights, np.float64)
    Vb, Ub = build_split_vu(rho, gamma, coords, weights)
    KK = Vb.shape[0]
    Ub16 = np.ascontiguousarray(Ub.astype(ml_dtypes.bfloat16))
    in_maps = []
    for m in range(N_CORES):
        vc = Vb[:, m * ROWS_PER_CORE:(m + 1) * ROWS_PER_CORE, :]  # [K, 2048, 3]
        vc = np.moveaxis(vc, 2, 1).reshape(KK, IC_PER_CORE)       # c-major cols
        in_maps.append({"u": Ub16,
                        "v": np.ascontiguousarray(vc.astype(ml_dtypes.bfloat16))})
    return KK, in_maps


def _assemble(results):
    out = np.empty((N, 3), np.float32)
    for m, res in enumerate(results):
        y_dev = np.asarray(res["y"])                   # [128, 48]
        flat = y_dev.T.reshape(IC_PER_CORE)            # ic = B*128 + p order
        out[m * ROWS_PER_CORE:(m + 1) * ROWS_PER_CORE, :] = (
            flat.reshape(3, ROWS_PER_CORE).T)
    return out


def kernel_run(rho, gamma, coords, weights, **spmd_kwargs):
    """Run on hardware; returns (y, BassKernelResults)."""
    from concourse.bass_utils import run_bass_kernel_spmd

    KK, in_maps = _prep_inputs(rho, gamma, coords, weights)
    if KK not in _NC_CACHE:
        _NC_CACHE[KK] = _build_nc(KK)
    res = run_bass_kernel_spmd(_NC_CACHE[KK], in_maps,
                               core_ids=list(range(N_CORES)), **spmd_kwargs)
    return _assemble(res.results), res


def kernel(rho, gamma, coords, weights):
    y, _ = kernel_run(rho, gamma, coords, weights)
    return y


# revision 12
# speedup vs baseline: 1.0044x; 1.0014x over previous
"""Trainium2 Bass kernel for nn_CiderFeatures (all-pairs Gaussian reduction).

y[i, c] = norms[c] * sum_j exp(-(a_j + b[i,c]) * ||x_i - x_j||^2) * f_j

with per-point scalars a, b, f derived from (rho, gamma, weights).

Strategy (8 NeuronCores, row-parallel over i):
  - The exp argument is a bilinear form: arg[ic, j] = V[:, ic] . U[:, j]
    with 10 logical contraction dims (the expansion of
    -(a_j + b_ic) (r_i + r_j - 2 x_i.x_j) + ln f_j + ln norms_c).
  - fp32 matmuls run at 1/4 rate on the PE, so each logical dim is split
    into bf16 hi/mid/lo levels and the cross products are stacked into the
    contraction dim (K~50..90).  bf16*bf16 products are exact in fp32
    accumulation, recovering ~fp32 precision at full PE speed.
  - TensorE computes arg tiles [128 ic, 512 j] into PSUM; ScalarE (ACT)
    computes exp and the j-sum in one pass via accum_out; VectorE adds the
    per-chunk partial sums.  ACT is the bottleneck (~3N^2/8 exps per core).
"""

import numpy as np
import ml_dtypes
from math import pi

N = 16384
N_CORES = 8
ROWS_PER_CORE = N // N_CORES          # 2048
IC_PER_CORE = 3 * ROWS_PER_CORE       # 6144
BLOCKS_PER_CORE = IC_PER_CORE // 128  # 48
J_GROUP = 2048                        # PSUM tile free size (4 banks)
N_GROUPS = N // J_GROUP               # 8
MM_N = 512                            # one PSUM bank of fp32
LNF_FLOOR = -100.0                    # ln f clamp for f == 0

# number of bf16 levels per operand and max level-sum kept
SPLIT_LEVELS = 3
MAX_LEVEL_SUM = 2


def _derived(rho, gamma, weights, coords):
    """Per-point scalars, computed in float64 (mirrors reference fp32 math)."""
    A, D = 2.0, 2.0
    B2, C2 = A, (6.0 * pi ** 2) ** (2.0 / 3.0) * (6.0 * A / (160.0 * pi))
    B3, C3 = 2.0 * B2, 2.0 * C2
    B0, C0 = D / A * B2, D / A * C2
    B1, C1 = B2 / 2.0, C2 / 2.0
    Bs = np.array([B0, B1, B2, B3])
    Cs = np.array([C0, C1, C2, C3])
    norms = ((Bs[0] + Bs[1:]) / 2.0) ** 1.5  # (3,)

    rho_ = rho + 1e-8
    t_w = gamma / (8.0 * rho_)
    t_tf = 0.3 * (3.0 * pi ** 2) ** (2.0 / 3.0) * rho_ ** (5.0 / 3.0)
    x = t_w / t_tf
    scale = pi * (rho_ / 2.0) ** (2.0 / 3.0)
    ab = scale[:, None] * (Bs[None, :] + Cs[None, :] * x[:, None])  # (N,4)
    a = ab[:, 0]
    b = ab[:, 1:]                                                   # (N,3)
    f = weights * rho
    lnf = np.log(np.maximum(f, 1e-300))
    lnf = np.maximum(lnf, LNF_FLOOR)
    r = np.sum(coords * coords, axis=1)                             # (N,)
    return a, b, f, lnf, r, norms


def _build_vu10(rho, gamma, coords, weights):
    """The 10-dim bilinear decomposition (float64).

    Returns V10 [10, N, 3] (per (i, c)) and U10 [10, N] (per j) with
      arg[ic, j] = sum_k V10[k, i, c] * U10[k, j]
                 = -(a_j + b_ic) * ||x_i - x_j||^2 + ln f_j + ln norms_c
    a and r are mean-centered to shrink cross-product magnitudes (the
    centered remainders fold into the pure-i / pure-j dims exactly).
    """
    a, b, f, lnf, r, norms = _derived(rho, gamma, weights, coords)
    lnn = np.log(norms)                                   # (3,)
    rbar = float(r.mean())
    rc = r - rbar
    abar = float(a.mean())
    ac = a - abar
    xyz = coords                                          # (N, 3)

    V10 = np.empty((10, N, 3))
    U10 = np.empty((10, N))

    # dim0: cross  -ac_j * rc_i
    V10[0] = np.broadcast_to(rc[:, None], (N, 3))
    U10[0] = -ac
    # dim1: pure j  (-a_j r_j + lnf_j - ac_j rbar)
    V10[1] = 1.0
    U10[1] = -a * r + lnf - ac * rbar
    # dims2-4: cross  2 x_i . (ac_j x_j)
    V10[2:5] = np.broadcast_to((2.0 * xyz).T[:, :, None], (3, N, 3))
    U10[2:5] = (ac[:, None] * xyz).T
    # dim5: cross  -b_ic * rc_j
    V10[5] = b
    U10[5] = -rc
    # dim6: pure ic  (-b_ic (r_i + rbar) - abar (rc_i + rbar) + lnn_c)
    V10[6] = (-(b * (r[:, None] + rbar))
              - abar * (rc[:, None] + rbar)
              + lnn[None, :])
    U10[6] = 1.0
    # dims7-9: cross  2 (b_ic + abar) x_i . x_j
    V10[7:10] = np.moveaxis(
        2.0 * (b + abar)[:, :, None] * xyz[:, None, :], 2, 0)
    U10[7:10] = xyz.T
    return V10, U10


def _bf16_levels(M, nlev):
    """Split float64 array into bf16-representable float64 level arrays."""
    rem = M.copy()
    outs = []
    for _ in range(nlev):
        h = np.asarray(rem, ml_dtypes.bfloat16).astype(np.float64)
        outs.append(h)
        rem = rem - h
    return outs


def build_split_vu(rho, gamma, coords, weights,
                   nlev=SPLIT_LEVELS, max_sum=MAX_LEVEL_SUM):
    """Build the bf16-split V/U matrices.

    Returns (Vb [K, N, 3], Ub [K, N]) float32 arrays whose values are
    bf16-representable, with  arg ~= sum_k Vb[k] * Ub[k]  accumulated in
    fp32.  Rows are ordered by level-sum (hi*hi products first).
    """
    V10, U10 = _build_vu10(rho, gamma, coords, weights)
    Vlev = [_bf16_levels(V10[d], nlev) for d in range(10)]
    Ulev = [_bf16_levels(U10[d], nlev) for d in range(10)]

    vrows, urows = [], []
    for s in range(max_sum + 1):
        for d in range(10):
            for lv in range(min(s, nlev - 1) + 1):
                lu = s - lv
                if lu >= nlev:
                    continue
                v = Vlev[d][lv]
                u = Ulev[d][lu]
                if not v.any() or not u.any():
                    continue
                vrows.append(v)
                urows.append(u)
    Vb = np.stack(vrows).astype(np.float32)   # [K, N, 3]
    Ub = np.stack(urows).astype(np.float32)   # [K, N]
    return Vb, Ub


# ---------------------------------------------------------------------------
# Device kernel
# ---------------------------------------------------------------------------

_NC_CACHE = {}


def _build_nc(KK, repeat=1):
    """One-core Bass program (SPMD across 8 cores with per-core inputs).

    repeat > 1 re-runs the whole compute loop (for benchmarking slope)."""
    import concourse.bass as bass  # noqa: F401
    import concourse.tile as tile
    from concourse import bacc, mybir

    nc = bacc.Bacc("TRN2", target_bir_lowering=False)
    u_dram = nc.dram_tensor("u", [KK, N], mybir.dt.bfloat16,
                            kind="ExternalInput")
    v_dram = nc.dram_tensor("v", [KK, IC_PER_CORE], mybir.dt.bfloat16,
                            kind="ExternalInput")
    y_dram = nc.dram_tensor("y", [128, BLOCKS_PER_CORE], mybir.dt.float32,
                            kind="ExternalOutput")

    # groups whose j-reduction runs on VectorE (reading fp32 exp scratch)
    # instead of ACT accum_out; the 6,7,7 block pattern balances the ACT and
    # DVE engine-busy times (both ~93% occupied).
    DVE_SETS = (frozenset((0, 1, 2, 4, 5, 6)),
                frozenset((0, 1, 2, 3, 4, 5, 6)),
                frozenset((0, 1, 2, 3, 4, 5, 6)))

    with tile.TileContext(nc) as tc:
        with (
            tc.tile_pool(name="singles", bufs=1) as singles,
            tc.tile_pool(name="upool", bufs=N_GROUPS) as upool,
            tc.tile_pool(name="psum", bufs=2, space="PSUM") as psum_pool,
            tc.tile_pool(name="scratch", bufs=3) as scratch_pool,
            tc.tile_pool(name="parts", bufs=3) as parts_pool,
        ):
            # warm the ACT exp table during the input-DMA window
            warm = singles.tile([128, 1], mybir.dt.float32)
            nc.vector.memset(warm[:], 0.0)
            nc.scalar.activation(out=warm[:], in_=warm[:],
                                 func=mybir.ActivationFunctionType.Exp)

            v_sb = singles.tile([KK, IC_PER_CORE], mybir.dt.bfloat16)
            nc.sync.dma_start(v_sb[:], v_dram[:])
            u_tiles = []
            for g in range(N_GROUPS):
                ut = upool.tile([KK, J_GROUP], mybir.dt.bfloat16, tag="u")
                nc.sync.dma_start(ut[:], u_dram[:, g * J_GROUP:(g + 1) * J_GROUP])
                u_tiles.append(ut)
            y_sb = singles.tile([128, BLOCKS_PER_CORE], mybir.dt.float32)

            for B in [b for _ in range(repeat) for b in range(BLOCKS_PER_CORE)]:
                lhsT = v_sb[:, B * 128:(B + 1) * 128]
                dve_groups = DVE_SETS[B % 3]
                parts = parts_pool.tile([128, N_GROUPS], mybir.dt.float32,
                                        tag="parts")
                for g in range(N_GROUPS):
                    pt = psum_pool.tile([128, J_GROUP], mybir.dt.float32,
                                        tag="ps")
                    for q in range(J_GROUP // MM_N):
                        nc.tensor.matmul(
                            pt[:, q * MM_N:(q + 1) * MM_N],
                            lhsT,
                            u_tiles[g][:, q * MM_N:(q + 1) * MM_N],
                            start=True, stop=True)
                    if g in dve_groups:
                        sc = scratch_pool.tile([128, J_GROUP],
                                               mybir.dt.float32, tag="sc")
                        nc.scalar.activation(
                            out=sc[:], in_=pt[:],
                            func=mybir.ActivationFunctionType.Exp)
                        nc.vector.reduce_sum(parts[:, g:g + 1], sc[:],
                                             axis=mybir.AxisListType.X)
                    else:
                        # exp in place in PSUM (ScalarE's cheapest port),
                        # j-sum via the ACT accumulator
                        nc.scalar.activation(
                            out=pt[:], in_=pt[:],
                            func=mybir.ActivationFunctionType.Exp,
                            accum_out=parts[:, g:g + 1])
                nc.vector.reduce_sum(y_sb[:, B:B + 1], parts[:],
                                     axis=mybir.AxisListType.X)
            nc.sync.dma_start(y_dram[:], y_sb[:])
    nc.finalize()
    return nc


def _prep_inputs(rho, gamma, coords, weights):
    rho = np.asarray(rho, np.float64)
    gamma = np.asarray(gamma, np.float64)
    coords = np.asarray(coords, np.float64)
    weights = np.asarray(weights, np.float64)
    Vb, Ub = build_split_vu(rho, gamma, coords, weights)
    KK = Vb.shape[0]
    Ub16 = np.ascontiguousarray(Ub.astype(ml_dtypes.bfloat16))
    in_maps = []
    for m in range(N_CORES):
        vc = Vb[:, m * ROWS_PER_CORE:(m + 1) * ROWS_PER_CORE, :]  # [K, 2048, 3]
        vc = np.moveaxis(vc, 2, 1).reshape(KK, IC_PER_CORE)       # c-major cols
        in_maps.append({"u": Ub16,
                        "v": np.ascontiguousarray(vc.astype(ml_dtypes.bfloat16))})
    return KK, in_maps


def _assemble(results):
    out = np.empty((N, 3), np.float32)
    for m, res in enumerate(results):
        y_dev = np.asarray(res["y"])                   # [128, 48]
        flat = y_dev.T.reshape(IC_PER_CORE)            # ic = B*128 + p order
        out[m * ROWS_PER_CORE:(m + 1) * ROWS_PER_CORE, :] = (
            flat.reshape(3, ROWS_PER_CORE).T)
    return out


def kernel_run(rho, gamma, coords, weights, **spmd_kwargs):
    """Run on hardware; returns (y, BassKernelResults)."""
    from concourse.bass_utils import run_bass_kernel_spmd

    KK, in_maps = _prep_inputs(rho, gamma, coords, weights)
    if KK not in _NC_CACHE:
        _NC_CACHE[KK] = _build_nc(KK)
    res = run_bass_kernel_spmd(_NC_CACHE[KK], in_maps,
                               core_ids=list(range(N_CORES)), **spmd_kwargs)
    return _assemble(res.results), res


def kernel(rho, gamma, coords, weights):
    y, _ = kernel_run(rho, gamma, coords, weights)
    return y


# revision 13
# speedup vs baseline: 1.0050x; 1.0006x over previous
"""Trainium2 Bass kernel for nn_CiderFeatures (all-pairs Gaussian reduction).

y[i, c] = norms[c] * sum_j exp(-(a_j + b[i,c]) * ||x_i - x_j||^2) * f_j

with per-point scalars a, b, f derived from (rho, gamma, weights).

Strategy (8 NeuronCores, row-parallel over i):
  - The exp argument is a bilinear form: arg[ic, j] = V[:, ic] . U[:, j]
    with 10 logical contraction dims (the expansion of
    -(a_j + b_ic) (r_i + r_j - 2 x_i.x_j) + ln f_j + ln norms_c).
  - fp32 matmuls run at 1/4 rate on the PE, so each logical dim is split
    into bf16 hi/mid/lo levels and the cross products are stacked into the
    contraction dim (K~50..90).  bf16*bf16 products are exact in fp32
    accumulation, recovering ~fp32 precision at full PE speed.
  - TensorE computes arg tiles [128 ic, 512 j] into PSUM; ScalarE (ACT)
    computes exp and the j-sum in one pass via accum_out; VectorE adds the
    per-chunk partial sums.  ACT is the bottleneck (~3N^2/8 exps per core).
"""

import numpy as np
import ml_dtypes
from math import pi

N = 16384
N_CORES = 8
ROWS_PER_CORE = N // N_CORES          # 2048
IC_PER_CORE = 3 * ROWS_PER_CORE       # 6144
BLOCKS_PER_CORE = IC_PER_CORE // 128  # 48
J_GROUP = 2048                        # PSUM tile free size (4 banks)
N_GROUPS = N // J_GROUP               # 8
MM_N = 512                            # one PSUM bank of fp32
LNF_FLOOR = -100.0                    # ln f clamp for f == 0

# number of bf16 levels per operand and max level-sum kept
SPLIT_LEVELS = 3
MAX_LEVEL_SUM = 2


def _derived(rho, gamma, weights, coords):
    """Per-point scalars, computed in float64 (mirrors reference fp32 math)."""
    A, D = 2.0, 2.0
    B2, C2 = A, (6.0 * pi ** 2) ** (2.0 / 3.0) * (6.0 * A / (160.0 * pi))
    B3, C3 = 2.0 * B2, 2.0 * C2
    B0, C0 = D / A * B2, D / A * C2
    B1, C1 = B2 / 2.0, C2 / 2.0
    Bs = np.array([B0, B1, B2, B3])
    Cs = np.array([C0, C1, C2, C3])
    norms = ((Bs[0] + Bs[1:]) / 2.0) ** 1.5  # (3,)

    rho_ = rho + 1e-8
    t_w = gamma / (8.0 * rho_)
    t_tf = 0.3 * (3.0 * pi ** 2) ** (2.0 / 3.0) * rho_ ** (5.0 / 3.0)
    x = t_w / t_tf
    scale = pi * (rho_ / 2.0) ** (2.0 / 3.0)
    ab = scale[:, None] * (Bs[None, :] + Cs[None, :] * x[:, None])  # (N,4)
    a = ab[:, 0]
    b = ab[:, 1:]                                                   # (N,3)
    f = weights * rho
    lnf = np.log(np.maximum(f, 1e-300))
    lnf = np.maximum(lnf, LNF_FLOOR)
    r = np.sum(coords * coords, axis=1)                             # (N,)
    return a, b, f, lnf, r, norms


def _build_vu10(rho, gamma, coords, weights):
    """The 10-dim bilinear decomposition (float64).

    Returns V10 [10, N, 3] (per (i, c)) and U10 [10, N] (per j) with
      arg[ic, j] = sum_k V10[k, i, c] * U10[k, j]
                 = -(a_j + b_ic) * ||x_i - x_j||^2 + ln f_j + ln norms_c
    a and r are mean-centered to shrink cross-product magnitudes (the
    centered remainders fold into the pure-i / pure-j dims exactly).
    """
    a, b, f, lnf, r, norms = _derived(rho, gamma, weights, coords)
    lnn = np.log(norms)                                   # (3,)
    rbar = float(r.mean())
    rc = r - rbar
    abar = float(a.mean())
    ac = a - abar
    xyz = coords                                          # (N, 3)

    V10 = np.empty((10, N, 3))
    U10 = np.empty((10, N))

    # dim0: cross  -ac_j * rc_i
    V10[0] = np.broadcast_to(rc[:, None], (N, 3))
    U10[0] = -ac
    # dim1: pure j  (-a_j r_j + lnf_j - ac_j rbar)
    V10[1] = 1.0
    U10[1] = -a * r + lnf - ac * rbar
    # dims2-4: cross  2 x_i . (ac_j x_j)
    V10[2:5] = np.broadcast_to((2.0 * xyz).T[:, :, None], (3, N, 3))
    U10[2:5] = (ac[:, None] * xyz).T
    # dim5: cross  -b_ic * rc_j
    V10[5] = b
    U10[5] = -rc
    # dim6: pure ic  (-b_ic (r_i + rbar) - abar (rc_i + rbar) + lnn_c)
    V10[6] = (-(b * (r[:, None] + rbar))
              - abar * (rc[:, None] + rbar)
              + lnn[None, :])
    U10[6] = 1.0
    # dims7-9: cross  2 (b_ic + abar) x_i . x_j
    V10[7:10] = np.moveaxis(
        2.0 * (b + abar)[:, :, None] * xyz[:, None, :], 2, 0)
    U10[7:10] = xyz.T
    return V10, U10


def _bf16_levels(M, nlev):
    """Split float64 array into bf16-representable float64 level arrays."""
    rem = M.copy()
    outs = []
    for _ in range(nlev):
        h = np.asarray(rem, ml_dtypes.bfloat16).astype(np.float64)
        outs.append(h)
        rem = rem - h
    return outs


def build_split_vu(rho, gamma, coords, weights,
                   nlev=SPLIT_LEVELS, max_sum=MAX_LEVEL_SUM):
    """Build the bf16-split V/U matrices.

    Returns (Vb [K, N, 3], Ub [K, N]) float32 arrays whose values are
    bf16-representable, with  arg ~= sum_k Vb[k] * Ub[k]  accumulated in
    fp32.  Rows are ordered by level-sum (hi*hi products first).
    """
    V10, U10 = _build_vu10(rho, gamma, coords, weights)
    Vlev = [_bf16_levels(V10[d], nlev) for d in range(10)]
    Ulev = [_bf16_levels(U10[d], nlev) for d in range(10)]

    vrows, urows = [], []
    for s in range(max_sum + 1):
        for d in range(10):
            for lv in range(min(s, nlev - 1) + 1):
                lu = s - lv
                if lu >= nlev:
                    continue
                v = Vlev[d][lv]
                u = Ulev[d][lu]
                if not v.any() or not u.any():
                    continue
                vrows.append(v)
                urows.append(u)
    Vb = np.stack(vrows).astype(np.float32)   # [K, N, 3]
    Ub = np.stack(urows).astype(np.float32)   # [K, N]
    return Vb, Ub


# ---------------------------------------------------------------------------
# Device kernel
# ---------------------------------------------------------------------------

_NC_CACHE = {}


def _build_nc(KK, repeat=1):
    """One-core Bass program (SPMD across 8 cores with per-core inputs).

    repeat > 1 re-runs the whole compute loop (for benchmarking slope)."""
    import concourse.bass as bass  # noqa: F401
    import concourse.tile as tile
    from concourse import bacc, mybir

    nc = bacc.Bacc("TRN2", target_bir_lowering=False)
    u_dram = nc.dram_tensor("u", [KK, N], mybir.dt.bfloat16,
                            kind="ExternalInput")
    v_dram = nc.dram_tensor("v", [KK, IC_PER_CORE], mybir.dt.bfloat16,
                            kind="ExternalInput")
    y_dram = nc.dram_tensor("y", [128, BLOCKS_PER_CORE], mybir.dt.float32,
                            kind="ExternalOutput")

    # groups whose j-reduction runs on VectorE (reading fp32 exp scratch)
    # instead of ACT accum_out; the 6,7,7,7 block pattern balances the ACT
    # and DVE engine-busy times (both ~93% occupied).
    DVE_SETS = (frozenset((0, 1, 2, 4, 5, 6)),
                frozenset((0, 1, 2, 3, 4, 5, 6)),
                frozenset((0, 1, 2, 3, 4, 5, 6)),
                frozenset((0, 1, 2, 3, 4, 5, 6)))

    with tile.TileContext(nc) as tc:
        with (
            tc.tile_pool(name="singles", bufs=1) as singles,
            tc.tile_pool(name="upool", bufs=N_GROUPS) as upool,
            tc.tile_pool(name="psum", bufs=2, space="PSUM") as psum_pool,
            tc.tile_pool(name="scratch", bufs=3) as scratch_pool,
            tc.tile_pool(name="parts", bufs=3) as parts_pool,
        ):
            # warm the ACT exp table during the input-DMA window
            warm = singles.tile([128, 1], mybir.dt.float32)
            nc.vector.memset(warm[:], 0.0)
            nc.scalar.activation(out=warm[:], in_=warm[:],
                                 func=mybir.ActivationFunctionType.Exp)

            v_sb = singles.tile([KK, IC_PER_CORE], mybir.dt.bfloat16)
            nc.sync.dma_start(v_sb[:], v_dram[:])
            u_tiles = []
            for g in range(N_GROUPS):
                ut = upool.tile([KK, J_GROUP], mybir.dt.bfloat16, tag="u")
                nc.sync.dma_start(ut[:], u_dram[:, g * J_GROUP:(g + 1) * J_GROUP])
                u_tiles.append(ut)
            y_sb = singles.tile([128, BLOCKS_PER_CORE], mybir.dt.float32)

            for B in [b for _ in range(repeat) for b in range(BLOCKS_PER_CORE)]:
                lhsT = v_sb[:, B * 128:(B + 1) * 128]
                dve_groups = DVE_SETS[B % 4]
                parts = parts_pool.tile([128, N_GROUPS], mybir.dt.float32,
                                        tag="parts")
                for g in range(N_GROUPS):
                    pt = psum_pool.tile([128, J_GROUP], mybir.dt.float32,
                                        tag="ps")
                    for q in range(J_GROUP // MM_N):
                        nc.tensor.matmul(
                            pt[:, q * MM_N:(q + 1) * MM_N],
                            lhsT,
                            u_tiles[g][:, q * MM_N:(q + 1) * MM_N],
                            start=True, stop=True)
                    if g in dve_groups:
                        sc = scratch_pool.tile([128, J_GROUP],
                                               mybir.dt.float32, tag="sc")
                        nc.scalar.activation(
                            out=sc[:], in_=pt[:],
                            func=mybir.ActivationFunctionType.Exp)
                        nc.vector.reduce_sum(parts[:, g:g + 1], sc[:],
                                             axis=mybir.AxisListType.X)
                    else:
                        # exp in place in PSUM (ScalarE's cheapest port),
                        # j-sum via the ACT accumulator
                        nc.scalar.activation(
                            out=pt[:], in_=pt[:],
                            func=mybir.ActivationFunctionType.Exp,
                            accum_out=parts[:, g:g + 1])
                nc.vector.reduce_sum(y_sb[:, B:B + 1], parts[:],
                                     axis=mybir.AxisListType.X)
            nc.sync.dma_start(y_dram[:], y_sb[:])
    nc.finalize()
    return nc


def _prep_inputs(rho, gamma, coords, weights):
    rho = np.asarray(rho, np.float64)
    gamma = np.asarray(gamma, np.float64)
    coords = np.asarray(coords, np.float64)
    weights = np.asarray(weights, np.float64)
    Vb, Ub = build_split_vu(rho, gamma, coords, weights)
    KK = Vb.shape[0]
    Ub16 = np.ascontiguousarray(Ub.astype(ml_dtypes.bfloat16))
    in_maps = []
    for m in range(N_CORES):
        vc = Vb[:, m * ROWS_PER_CORE:(m + 1) * ROWS_PER_CORE, :]  # [K, 2048, 3]
        vc = np.moveaxis(vc, 2, 1).reshape(KK, IC_PER_CORE)       # c-major cols
        in_maps.append({"u": Ub16,
                        "v": np.ascontiguousarray(vc.astype(ml_dtypes.bfloat16))})
    return KK, in_maps


def _assemble(results):
    out = np.empty((N, 3), np.float32)
    for m, res in enumerate(results):
        y_dev = np.asarray(res["y"])                   # [128, 48]
        flat = y_dev.T.reshape(IC_PER_CORE)            # ic = B*128 + p order
        out[m * ROWS_PER_CORE:(m + 1) * ROWS_PER_CORE, :] = (
            flat.reshape(3, ROWS_PER_CORE).T)
    return out


def kernel_run(rho, gamma, coords, weights, **spmd_kwargs):
    """Run on hardware; returns (y, BassKernelResults)."""
    from concourse.bass_utils import run_bass_kernel_spmd

    KK, in_maps = _prep_inputs(rho, gamma, coords, weights)
    if KK not in _NC_CACHE:
        _NC_CACHE[KK] = _build_nc(KK)
    res = run_bass_kernel_spmd(_NC_CACHE[KK], in_maps,
                               core_ids=list(range(N_CORES)), **spmd_kwargs)
    return _assemble(res.results), res


def kernel(rho, gamma, coords, weights):
    y, _ = kernel_run(rho, gamma, coords, weights)
    return y
